# revision 23
# baseline (speedup 1.0000x reference)
"""Trainium2 Bass kernel for nn_MultiHeadAttention_88192858456426.

Reference computation (per batch, C=512 channels, N=2048 tokens):
    qp = Wq q + bq 1^T;  kp = Wk k + bk 1^T;  vp = Wv v + bv 1^T   # [C, N]
    out = vp (kp^T qp) + q                                          # [C, N]

There is no softmax, so the product reassociates: out = M qp + q with
M = vp kp^T in [C, C].  Expanding the projections,

    M   = Wv A^T Wk^T + u bk^T + bv w'^T          A  = k v^T   (Gram, CxC)
    G^T = Wq^T M^T = (Wk^T Wq)^T ... computed as  U = A^T (Wk^T Wq)
          G^T = U^T Wv^T + (Wq^T bk) u^T + (Wq^T w') bv^T
    out = (G + I) q + (M bq) 1^T                  (I folds the residual)

with u = Wv (v 1), w' = Wk (k 1) + N bk.  This needs one [C,C] Gram matmul
over N (32.7k PE cycles), two C^3 matmuls (16.4k), the final G q (32.7k)
and some rank-1/matvec crumbs -- ~87k PE cycles/core vs ~360k for the
direct qp/kp/vp dataflow.  Data-parallel over batch B=8, one batch per
core, no collectives.  All matmul operands fp16 (PSUM accumulates fp32);
host precomputes transposes/weight-products (Wk^T Wq etc.) and the
token-sum correction vectors.

Device dataflow (all matmuls out[M,Nf] = lhsT[K,M].T @ rhs[K,Nf]):
  A[a,b]   : lhsT = kT[n, a-chunk], rhs = vT[n, :]      acc over 16 n-chunks
  y[b]     = A^T g   (g = Wk^T bq, for Mbq)             tiny Nf=1 matmuls
  U[b,l]   : lhsT = A[a, b-chunk], rhs = P2T[a, :]      P2T = Wk^T Wq
  z[i]     = Wv y + mb0                                 tiny Nf=1 matmuls
  G^T[l,i] : lhsT = U[b, l-chunk], rhs = WvT[b, :]
             + corrGL^T corrGR (K=2) + I (identity matmul)
  out[i,n] : lhsT = G^T[l, i-chunk], rhs = q[l, n-blk]; ACT bias adds Mbq.
"""

import numpy as np
from contextlib import ExitStack

import concourse.bass as bass
import concourse.mybir as mybir
import concourse.tile as tile
from concourse import bacc
from concourse.bass_utils import run_bass_kernel_spmd

P = 128            # partitions
C = 512            # channels
N = 2048           # tokens
NB = 512           # n-block width (one PSUM bank of fp32)
CK = C // P        # 4 channel chunks
NCH = N // P       # 16 token chunks
NBK = N // NB      # 4 n-blocks

F32 = mybir.dt.float32
FP16 = mybir.dt.float16
ACT_IDENT = mybir.ActivationFunctionType.Identity

N_CORES = 8


def build_nc(reps=1, mode="fp16", timing=False, ablate=None):
    """timing=True keeps the [C,N] output in Internal DRAM and exposes a
    [1,1] dummy ExternalOutput instead -- the axon tunnel's per-call output
    fetch otherwise swamps rep-slope timing.  ablate in {"noload",
    "nostore"} builds diagnostic variants (timing only, results wrong)."""
    MDT = FP16
    nc = bacc.Bacc("TRN2", target_bir_lowering=False, debug=False,
                   num_devices=N_CORES)

    # timing NEFFs keep the bulk tensors device-side (Internal): the axon
    # tunnel re-ships every ExternalInput per call, which otherwise swamps
    # the measurement.  Instruction stream / bytes moved are identical.
    in_kind = "Internal" if timing else "ExternalInput"
    kT_d = nc.dram_tensor("kT", [N, C], MDT, kind=in_kind).ap()
    vT_d = nc.dram_tensor("vT", [N, C], MDT, kind=in_kind).ap()
    q_d = nc.dram_tensor("q", [C, N], MDT, kind=in_kind).ap()
    p2t_d = nc.dram_tensor("p2t", [C, C], MDT, kind=in_kind).ap()
    wvt_d = nc.dram_tensor("wvt", [C, C], MDT, kind=in_kind).ap()
    corrgl_d = nc.dram_tensor("corrgl", [2, C], MDT, kind="ExternalInput").ap()
    corrgr_d = nc.dram_tensor("corrgr", [2, C], MDT, kind="ExternalInput").ap()
    gvec_d = nc.dram_tensor("gvec", [P, CK], MDT, kind="ExternalInput").ap()
    mb0t_d = nc.dram_tensor("mb0t", [1, C], MDT, kind="ExternalInput").ap()
    one_d = nc.dram_tensor("one", [1, 1], MDT, kind="ExternalInput").ap()
    ident_d = nc.dram_tensor("ident", [P, P], MDT, kind=in_kind).ap()
    # output leaves the device as fp16 (half the store traffic; host
    # upcasts to f32 -- adds ~5e-4 relative error, well inside the gate)
    o_kind = "Internal" if timing else "ExternalOutput"
    o_d = nc.dram_tensor("o", [C, N], MDT, kind=o_kind).ap()
    t_d = (nc.dram_tensor("t", [1, 1], FP16, kind="ExternalOutput").ap()
           if timing else None)

    with ExitStack() as ctx:
        tc = ctx.enter_context(tile.TileContext(nc))
        kvpool = ctx.enter_context(tc.tile_pool(name="kvpool", bufs=1))
        qpool = ctx.enter_context(tc.tile_pool(name="qpool", bufs=1))
        wpool = ctx.enter_context(tc.tile_pool(name="wpool", bufs=1))
        consts = ctx.enter_context(tc.tile_pool(name="consts", bufs=1))
        abuf = ctx.enter_context(tc.tile_pool(name="abuf", bufs=1))
        opool = ctx.enter_context(tc.tile_pool(name="opool", bufs=4))
        ps_a = ctx.enter_context(tc.tile_pool(name="ps_a", bufs=4, space="PSUM"))
        ps_g = ctx.enter_context(tc.tile_pool(name="ps_g", bufs=2, space="PSUM"))
        ps_o = ctx.enter_context(tc.tile_pool(name="ps_o", bufs=2, space="PSUM"))

        # ---- weights / constants: loaded once, resident across reps ----
        p2t_sb = []
        for i in range(CK):
            t = wpool.tile([P, C], MDT, tag=f"p2t{i}", name=f"p2t{i}")
            nc.sync.dma_start(t[:], p2t_d[i * P:(i + 1) * P, :])
            p2t_sb.append(t)
        wvt_sb = []
        for i in range(CK):
            t = wpool.tile([P, C], MDT, tag=f"wvt{i}", name=f"wvt{i}")
            nc.sync.dma_start(t[:], wvt_d[i * P:(i + 1) * P, :])
            wvt_sb.append(t)
        gvec = consts.tile([P, CK], MDT, tag="gvec", name="gvec")
        nc.sync.dma_start(gvec[:], gvec_d[:])
        one_sb = consts.tile([1, 1], MDT, tag="one", name="one")
        nc.sync.dma_start(one_sb[:], one_d[:])
        ident = consts.tile([P, P], MDT, tag="ident", name="ident")
        nc.sync.dma_start(ident[:], ident_d[:])

        if ablate == "noload":
            # static tiles, memset once -- measures the DMA-free timeline
            kt_sb, vt_sb, q_sb = [], [], []
            for n in range(NCH):
                t = kvpool.tile([P, C], MDT, tag=f"kt{n}", name=f"kt{n}")
                nc.vector.memset(t[:], 0.25)
                kt_sb.append(t)
                t = kvpool.tile([P, C], MDT, tag=f"vt{n}", name=f"vt{n}")
                nc.vector.memset(t[:], 0.25)
                vt_sb.append(t)
            for l in range(CK):
                t = qpool.tile([P, N], MDT, tag=f"q{l}", name=f"q{l}")
                nc.vector.memset(t[:], 0.25)
                q_sb.append(t)
            corrgl_st = consts.tile([2, C], MDT, tag="corrgl", name="corrgl")
            nc.vector.memset(corrgl_st[:], 0.25)
            corrgr_st = consts.tile([2, C], MDT, tag="corrgr", name="corrgr")
            nc.vector.memset(corrgr_st[:], 0.25)
            mb0t_st = consts.tile([1, C], MDT, tag="mb0t", name="mb0t")
            nc.vector.memset(mb0t_st[:], 0.25)

        for rep in range(reps):
            # ---- per-batch data: kT/vT pairs stream on the sync ring in
            # the order phase A consumes them, then the small input-derived
            # correction tiles, then q (needed only by the out phase).
            # Stores ride the scalar ring, so in the rep loop the next
            # rep's kT/vT prefetch overlaps this rep's U/G/out phases.
            if ablate == "noload":
                corrgl, corrgr, mb0t = corrgl_st, corrgr_st, mb0t_st
            else:
                kt_sb, vt_sb = [], []
                for n in range(NCH):
                    t = kvpool.tile([P, C], MDT, tag=f"kt{n}", name=f"kt{n}")
                    nc.sync.dma_start(t[:], kT_d[n * P:(n + 1) * P, :])
                    kt_sb.append(t)
                    t = kvpool.tile([P, C], MDT, tag=f"vt{n}", name=f"vt{n}")
                    nc.sync.dma_start(t[:], vT_d[n * P:(n + 1) * P, :])
                    vt_sb.append(t)
                corrgl = consts.tile([2, C], MDT, tag="corrgl", name="corrgl")
                nc.sync.dma_start(corrgl[:], corrgl_d[:])
                corrgr = consts.tile([2, C], MDT, tag="corrgr", name="corrgr")
                nc.sync.dma_start(corrgr[:], corrgr_d[:])
                mb0t = consts.tile([1, C], MDT, tag="mb0t", name="mb0t")
                nc.sync.dma_start(mb0t[:], mb0t_d[:])
                # q rides the scalar ring ahead of the stores: the sync
                # ring then reaches the next rep's kT/vT sooner
                q_sb = []
                for l in range(CK):
                    t = qpool.tile([P, N], MDT, tag=f"q{l}", name=f"q{l}")
                    nc.scalar.dma_start(t[:], q_d[l * P:(l + 1) * P, :])
                    q_sb.append(t)

            # ---- phase A: A[a,b] = sum_n kT[n,a] vT[n,b] ----
            # n-outer so the PE consumes kT/vT pairs in DMA arrival order;
            # all four a-chunk accumulation groups stay live in PSUM.  The
            # last TAILN n-chunks run a-outer so chunk a's PSUM->SBUF copy
            # overlaps chunk a+1's remaining matmuls (no A->U bubble).
            TAILN = 2
            ps_A = [ps_a.tile([P, C], F32, tag="psa", name=f"psA{a}")
                    for a in range(CK)]
            for n in range(NCH - TAILN):
                for a in range(CK):
                    nc.tensor.matmul(
                        ps_A[a][:],
                        kt_sb[n][:, a * P:(a + 1) * P],
                        vt_sb[n][:],
                        start=(n == 0), stop=False)
            a_sb = []
            for a in range(CK):
                for n in range(NCH - TAILN, NCH):
                    nc.tensor.matmul(
                        ps_A[a][:],
                        kt_sb[n][:, a * P:(a + 1) * P],
                        vt_sb[n][:],
                        start=False, stop=(n == NCH - 1))
                t = abuf.tile([P, C], MDT, tag=f"a{a}", name=f"a{a}")
                if a % 2 == 0:
                    nc.scalar.copy(t[:], ps_A[a][:])
                else:
                    nc.vector.tensor_copy(t[:], ps_A[a][:])
                a_sb.append(t)

            # ---- U[b,l] = sum_a A[a,b] P2T[a,l], with y = A^T g woven in:
            # the y matmul for (b,a) reuses the U matmul's stationary
            # operand (same a_sb slice), so its weight load is free.
            ps_y = (None if ablate == "nocrumb" else
                    ps_a.tile([P, C], F32, tag="psa", name="psy"))
            u_sb = []
            for b in range(CK):
                ps = ps_g.tile([P, C], F32, tag="psg", name=f"psU{b}")
                for a in range(CK):
                    lhs = a_sb[a][:, b * P:(b + 1) * P]
                    nc.tensor.matmul(
                        ps[:], lhs, p2t_sb[a][:],
                        start=(a == 0), stop=(a == CK - 1))
                    if ablate != "nocrumb":
                        nc.tensor.matmul(
                            ps_y[:, b:b + 1], lhs, gvec[:, a:a + 1],
                            start=(a == 0), stop=(a == CK - 1),
                            skip_group_check=True)
                t = abuf.tile([P, C], MDT, tag=f"u{b}", name=f"u{b}")
                if b % 2 == 0:
                    nc.scalar.copy(t[:], ps[:])
                else:
                    nc.vector.tensor_copy(t[:], ps[:])
                u_sb.append(t)
            if ablate != "nocrumb":
                y_sb = consts.tile([P, CK], MDT, tag="y", name="y")
                nc.scalar.copy(y_sb[:], ps_y[:, 0:CK])

            # ---- z = Wv y + mb0 -> Mbq (ACT bias for the out phase) ----
            ps_z = (None if ablate == "nocrumb" else
                    ps_a.tile([P, C], F32, tag="psa", name="psz"))
            for i in (range(CK) if ablate != "nocrumb" else []):
                for b in range(CK):
                    nc.tensor.matmul(
                        ps_z[:, i:i + 1],
                        wvt_sb[b][:, i * P:(i + 1) * P],
                        y_sb[:, b:b + 1],
                        start=(b == 0), stop=False,
                        skip_group_check=True)
                nc.tensor.matmul(
                    ps_z[:, i:i + 1],
                    mb0t[0:1, i * P:(i + 1) * P],
                    one_sb[:],
                    start=False, stop=True,
                    skip_group_check=True)
            if ablate != "nocrumb":
                mbq_sb = consts.tile([P, CK], F32, tag="mbq", name="mbq")
                nc.scalar.copy(mbq_sb[:], ps_z[:, 0:CK])

            # ---- G'^T[l,i] = sum_b U[b,l] WvT[b,i] + corr + I ----
            gt_sb = []
            for l in range(CK):
                ps = ps_g.tile([P, C], F32, tag="psg", name=f"psG{l}")
                for b in range(CK):
                    nc.tensor.matmul(
                        ps[:],
                        u_sb[b][:, l * P:(l + 1) * P],
                        wvt_sb[b][:],
                        start=(b == 0), stop=False)
                if ablate != "nocrumb":
                    nc.tensor.matmul(
                        ps[:, l * P:(l + 1) * P],
                        ident[:], ident[:],
                        start=False, stop=False,
                        skip_group_check=True)
                    nc.tensor.matmul(
                        ps[:],
                        corrgl[:, l * P:(l + 1) * P],
                        corrgr[:],
                        start=False, stop=True,
                        skip_group_check=True)
                t = abuf.tile([P, C], MDT, tag=f"g{l}", name=f"g{l}")
                if l % 2 == 0:
                    nc.scalar.copy(t[:], ps[:])
                else:
                    nc.vector.tensor_copy(t[:], ps[:])
                gt_sb.append(t)

            # ---- out[i, nb] = sum_l G'^T[l,i] q[l, nb] + Mbq[i] ----
            for nb in range(NBK):
                for i in range(CK):
                    ps = ps_o.tile([P, NB], F32, tag="pso", name="pso")
                    for l in range(CK):
                        nc.tensor.matmul(
                            ps[:],
                            gt_sb[l][:, i * P:(i + 1) * P],
                            q_sb[l][:, nb * NB:(nb + 1) * NB],
                            start=(l == 0), stop=(l == CK - 1))
                    o_sb = opool.tile([P, NB], MDT, tag="o", name="o")
                    if ablate == "nocrumb":
                        if (nb * CK + i) % 2 == 0:
                            nc.scalar.copy(o_sb[:], ps[:])
                        else:
                            nc.vector.tensor_copy(o_sb[:], ps[:])
                    elif (nb * CK + i) % 2 == 0:
                        nc.scalar.activation(o_sb[:], ps[:], ACT_IDENT,
                                             bias=mbq_sb[:, i:i + 1])
                    else:
                        nc.vector.tensor_scalar_add(o_sb[:], ps[:],
                                                    mbq_sb[:, i:i + 1])
                    # stores go on the scalar ring, keeping the sync ring
                    # free for the next rep's kT/vT prefetch
                    if ablate != "nostore":
                        nc.scalar.dma_start(o_d[i * P:(i + 1) * P,
                                                nb * NB:(nb + 1) * NB],
                                            o_sb[:])

        if timing:
            nc.sync.dma_start(t_d[:], o_sb[0:1, 0:1])

    nc.finalize()
    return nc


_CACHE = {}


MODE = "fp16"


def _get_nc():
    if "nc" not in _CACHE:
        _CACHE["nc"] = build_nc(mode=MODE)
    return _CACHE["nc"]


def _in_maps(q, k, v, wq, bq, wk, bk, wv, bv, mode=None):
    f16 = lambda x: np.ascontiguousarray(np.asarray(x, dtype=np.float32)
                                         .astype(np.float16))
    q32 = np.asarray(q, np.float32)
    k32 = np.asarray(k, np.float32)
    v32 = np.asarray(v, np.float32)
    wq32 = np.asarray(wq, np.float32)
    wk32 = np.asarray(wk, np.float32)
    wv32 = np.asarray(wv, np.float32)
    bq32 = np.asarray(bq, np.float32)
    bk32 = np.asarray(bk, np.float32)
    bv32 = np.asarray(bv, np.float32)

    p2t = f16(wk32.T @ wq32)                 # [a, l] = (Wq^T Wk)^T
    wvt = f16(wv32.T)                        # [b, i]
    g = wk32.T @ bq32
    gvec = f16(g.reshape(CK, P).T)           # [128, 4], col a = chunk a
    ident = f16(np.eye(P, dtype=np.float32))
    one = f16(np.ones((1, 1), np.float32))
    wqTbk = wq32.T @ bk32
    s1 = float(bk32 @ bq32)

    maps = []
    for i in range(N_CORES):
        kb, vb, qb = k32[i], v32[i], q32[i]
        sv = vb.sum(1)
        sk = kb.sum(1)
        u = wv32 @ sv
        wp = wk32 @ sk + N * bk32
        s2 = float(wp @ bq32)
        mb0 = u * s1 + bv32 * s2
        corrgl = np.stack([wqTbk, wq32.T @ wp])    # [2, C] (l-index)
        corrgr = np.stack([u, bv32])               # [2, C] (i-index)
        maps.append({
            "kT": f16(kb.T), "vT": f16(vb.T), "q": f16(qb),
            "p2t": p2t, "wvt": wvt,
            "corrgl": f16(corrgl), "corrgr": f16(corrgr),
            "gvec": gvec, "mb0t": f16(mb0[None, :]),
            "one": one, "ident": ident,
        })
    return maps


def run(inputs, **spmd_kwargs):
    """Run on hardware; returns (output [B,C,N], BassKernelResults)."""
    nc = _get_nc()
    maps = _in_maps(**inputs)
    res = run_bass_kernel_spmd(nc, maps, list(range(N_CORES)), **spmd_kwargs)
    out = np.stack([res.results[i]["o"].astype(np.float32)
                    for i in range(N_CORES)], axis=0)
    return out, res


def kernel(q, k, v, wq, bq, wk, bk, wv, bv):
    out, _ = run(dict(q=q, k=k, v=v, wq=wq, bq=bq, wk=wk, bk=bk,
                      wv=wv, bv=bv))
    return out
